# revision 6
# baseline (speedup 1.0000x reference)
# Trainium2 Bass kernel for DeepFeatureKNN: exact k-NN (k<=16) of 4096 queries
# against 65536 database embeddings (D=256), sharded over 8 NeuronCores
# (database dim split 8 ways; every core sees all queries).
#
# The database is pre-sorted by ||e||^2 on the host, so within any block of
# 8 consecutive (sorted) rows the e2 spread is tiny. The device only computes
# 2*dots (GEMM) and per-group maxima of 2*dots; the host forms the provable
# upper bound ub(g) = gmax2dots(g) - min_{n in g} e2(n) >= v(x) - BETA for
# x in g (v = 2*dots - e2; descending v == ascending distance), selects the
# top-CG groups per query by ub, exactly rescores their member columns
# (fp32 narrowing + fp64), and picks the true top-k. Certificate: if every
# group with ub >= (kth exact v) - BETA made the top-CG cut, the candidate
# set provably contains the true top-k; failures fall back to an exact fp64
# full rescan of that query.
#
# Device-side structure (the performance-critical part):
#  - matmul: queries stationary (128 per tile), emb columns moving (512 per
#    PSUM bank). KNN_MM=fp16: two K=128 fp16 matmuls per bank. KNN_MM=fp8dr:
#    ONE fp8-e4m3 DoubleRow K=256 matmul per bank, with redundant LDWEIGHTS
#    deleted post-Tile so the 256-col (non-FWL) weight load happens once per
#    query tile instead of once per matmul (the naive form was measured
#    SLOWER than fp16 because of serialized LDWEIGHTS).
#  - group max: DVE tensor_reduce only has a 1x uop (1 elem/cyc @0.96GHz),
#    so reducing every PSUM value on DVE alone costs ~289us and was the
#    baseline bottleneck. The reduce is split across engines per 4-bank
#    group: 'D' banks reduce directly on DVE (tensor_reduce, 1 elem/cyc);
#    'S' banks are staged PSUM f32 -> SBUF bf16 by the Scalar/ACT engine
#    (1 elem/cyc @1.2GHz) and then max-combined on DVE with tensor_tensor
#    max (2x_1P mode on bf16 = 2 elem/cyc) in a tree.
#  - group geometry: groups are parity subsets of each 8-block so that the
#    TT tree and the direct reduce produce the SAME grouping: G=4 group j
#    (j in 0,1) of block b covers sorted cols {8b+j, 8b+j+2, 8b+j+4, 8b+j+6};
#    G=2 group j (j in 0..3) covers {8b+j, 8b+j+4}; G=8 is the whole block.

import os
import numpy as np

N, D, MQ = 65536, 256, 4096
NCORES = 8
NSH = N // NCORES       # 8192 database rows per core
QT = 128                # queries per partition tile
NT = 512                # database columns per tile (= one PSUM bank of fp32)
NTILES = NSH // NT      # 16
QTILES = MQ // QT       # 32
BG = 4                  # psum banks per bank-group (reduce granularity)
NBQ = NTILES // BG      # 4 bank-groups per query tile
KMAX = 16

# ---- configuration (env-overridable for A/B perf runs) ----
MM_MODE = os.environ.get("KNN_MM", "fp8dr")       # fp16 | fp8dr
G = int(os.environ.get("KNN_G", "4"))             # 8, 4, or 2 (cols/group)
# per-qt bank-group modes, comma-separated patterns cycled over qt:
# 'D' = DVE direct tensor_reduce, 'S' = ACT stage + DVE TT-max tree
PAT = os.environ.get("KNN_PAT", "DSSS,DDSS").split(",")
SPLIT_DMA = os.environ.get("KNN_SPLITDMA", "1") == "1"
DEDUP_LDW = os.environ.get("KNN_DEDUP", "1") == "1"

NG = NSH // G           # groups per core
PG = 8 // G             # groups per 8-block
if MM_MODE == "fp8dr":
    CGBASE = 80         # groups-of-8-equivalent rescored per query
    BETA = 9.0          # certificate band (device underestimate bound)
else:
    CGBASE = 48
    BETA = 0.35
CG = int(os.environ.get("KNN_CG", str(CGBASE * PG)))

_CACHE = {}


def _build_bass(repeat=None):
    import concourse.bacc as bacc
    import concourse.tile as tile
    import concourse.mybir as mybir

    f32 = mybir.dt.float32
    f16 = mybir.dt.float16
    bf16 = mybir.dt.bfloat16
    in_dt = mybir.dt.float8e4 if MM_MODE == "fp8dr" else f16
    DR = mybir.MatmulPerfMode.DoubleRow
    X = mybir.AxisListType.X
    MAX = mybir.AluOpType.max

    if repeat is None:
        repeat = int(os.environ.get("KNN_REPEAT", "1"))

    nc = bacc.Bacc("TRN2", target_bir_lowering=False, debug=False,
                   num_devices=NCORES)

    embT_d = nc.dram_tensor("embT", [2, 128, NSH], in_dt, kind="ExternalInput")
    subT_d = nc.dram_tensor("subT", [2, 128, MQ], in_dt, kind="ExternalInput")
    gmax_d = nc.dram_tensor("gmax", [MQ, NG], f16, kind="ExternalOutput")

    GOUT_BQ = BG * (NT // 8) * PG   # group-max outputs per bank-group

    with tile.TileContext(nc) as tc:
        with (
            tc.tile_pool(name="const", bufs=1) as const,
            tc.tile_pool(name="stg", bufs=3) as stg,
            tc.tile_pool(name="gmp", bufs=3) as gmp,
            tc.tile_pool(name="ps", bufs=2, space="PSUM") as psum,
        ):
            subT = const.tile([128, 2, MQ], in_dt)
            if SPLIT_DMA:
                # Fine-grained input loads spread over the DGE queues so the
                # first matmuls start ~6us in instead of waiting ~23us for
                # one monolithic transfer.
                engs = [nc.sync, nc.scalar, nc.gpsimd]
                ei = 0
                for c in range(2):
                    engs[ei % 3].dma_start(subT[:, c, 0:512],
                                           subT_d[c, :, 0:512])
                    ei += 1
                embN = []
                for t in range(NTILES):
                    et = const.tile([128, 2, NT], in_dt, name=f"embN{t}")
                    embN.append(et)
                for t in range(NTILES):
                    for c in range(2):
                        sl = slice(t * NT, (t + 1) * NT)
                        engs[ei % 3].dma_start(embN[t][:, c, :],
                                               embT_d[c, :, sl])
                        ei += 1
                    if t == 3:
                        for c in range(2):
                            for h in range(1, 8):
                                sl = slice(h * 512, (h + 1) * 512)
                                engs[ei % 3].dma_start(subT[:, c, sl],
                                                       subT_d[c, :, sl])
                                ei += 1
            else:
                embT = const.tile([128, 2, NSH], in_dt)
                for c in range(2):
                    nc.sync.dma_start(embT[:, c, :], embT_d[c])
                    nc.sync.dma_start(subT[:, c, :], subT_d[c])
                embN = [embT[:, :, t * NT:(t + 1) * NT] for t in range(NTILES)]

            if repeat > 1:
                loop_cm = tc.For_i(0, repeat, 1)
            else:
                import contextlib
                loop_cm = contextlib.nullcontext()

            with loop_cm:
                for qt in range(QTILES):
                    qs = slice(qt * QT, (qt + 1) * QT)
                    pat = PAT[qt % len(PAT)]
                    gm = gmp.tile([128, NBQ, GOUT_BQ], f16, tag="gm")
                    for bq in range(NBQ):
                        ps = psum.tile([128, BG, NT], f32, tag="ps")
                        if MM_MODE == "fp8dr":
                            # one DoubleRow K=256 matmul per bank; LDWEIGHTS
                            # dedup below makes the weight load once-per-qt
                            for j in range(BG):
                                nt = bq * BG + j
                                nc.tensor.matmul(ps[:, j], subT[:, :, qs],
                                                 embN[nt][:, :, :],
                                                 start=True, stop=True,
                                                 perf_mode=DR)
                        else:
                            # group matmuls by stationary operand (weight
                            # reuse): chunk 0 into all BG banks, then chunk 1
                            for c in range(2):
                                for j in range(BG):
                                    nt = bq * BG + j
                                    nc.tensor.matmul(ps[:, j], subT[:, c, qs],
                                                     embN[nt][:, c, :],
                                                     start=(c == 0),
                                                     stop=(c == 1))
                        mode = pat[bq % len(pat)]
                        gout = gm[:, bq].rearrange(
                            "p (b n j) -> p b n j", b=BG, n=NT // 8, j=PG)
                        if mode == "D":
                            # DVE direct reduce; strided views make parity
                            # groups that match the 'S' path's TT tree:
                            # group j of a block = cols {j, j+PG, j+2PG, ...}
                            src = ps[:].rearrange(
                                "p b (n k j) -> p b n j k", j=PG, k=G)
                            nc.vector.tensor_reduce(gout, src, axis=X, op=MAX)
                        else:
                            # ACT stages psum f32 -> SBUF bf16 (1 elem/cyc
                            # @1.2GHz), then DVE TT-max tree at 2 elem/cyc.
                            st = stg.tile([128, BG, NT // 8, 8], f16,
                                          tag="st")
                            nc.scalar.copy(st[:], ps[:])
                            if G == 2:
                                nc.vector.tensor_max(
                                    gout, st[:, :, :, 0:4], st[:, :, :, 4:8])
                            else:
                                t1 = stg.tile([128, BG, NT // 8, 4], f16,
                                              tag="t1")
                                nc.vector.tensor_max(
                                    t1[:], st[:, :, :, 0:4], st[:, :, :, 4:8])
                                if G == 4:
                                    nc.vector.tensor_max(
                                        gout, t1[:, :, :, 0:2],
                                        t1[:, :, :, 2:4])
                                else:  # G == 8
                                    t2 = stg.tile([128, BG, NT // 8, 2],
                                                  f16, tag="t2")
                                    nc.vector.tensor_max(
                                        t2[:], t1[:, :, :, 0:2],
                                        t1[:, :, :, 2:4])
                                    nc.vector.tensor_max(
                                        gout, t2[:, :, :, 0:1],
                                        t2[:, :, :, 1:2])
                    nc.sync.dma_start(gmax_d[qt * QT:(qt + 1) * QT, :], gm[:])
    if DEDUP_LDW:
        _dedup_ldweights(nc)
    nc.compile()
    return nc


def _dedup_ldweights(nc):
    """Delete InstLdweights whose weights signature matches the immediately
    preceding load (the PE array keeps the stationary operand resident).
    Only sync-free duplicates are removed; tracking resets at basic-block
    boundaries. This turns fp8 DoubleRow's per-matmul 256-col (non-FWL)
    weight reload into a once-per-query-tile load."""
    import concourse.mybir as mybir

    def wkey(i):
        ap = i.ins[0]
        return (repr(ap), getattr(i, "perf_mode", None),
                getattr(i, "tile_position", None),
                getattr(i, "tile_size", None))

    removed = 0
    for blk in nc.main_func.blocks:
        last = None
        keep = []
        for inst in blk.instructions:
            tn = type(inst).__name__
            if tn == "InstLdweights":
                si = inst.sync_info
                clean = si is None or (not si.on_wait and not si.on_update)
                k = wkey(inst)
                if clean and last is not None and k == last:
                    removed += 1
                    continue
                last = k
            keep.append(inst)
        if removed:
            del blk.instructions[:]
            for inst in keep:
                blk.instructions.append(inst)
    return removed


def _colmap():
    """[N//G, G] int64: sorted-space column indices of each device group."""
    blocks = np.arange(N // 8, dtype=np.int64)[:, None, None]    # [NB,1,1]
    j = np.arange(PG, dtype=np.int64)[None, :, None]             # [1,PG,1]
    k = np.arange(G, dtype=np.int64)[None, None, :]              # [1,1,G]
    cols = 8 * blocks + j + PG * k                               # [NB,PG,G]
    return cols.reshape(N // G, G)


def _prep_inputs(emb: np.ndarray, sub: np.ndarray):
    # sort database rows by ||e||^2 so each 8-block has ~zero e2 spread
    e2_64 = (emb.astype(np.float64) ** 2).sum(-1)          # [N]
    perm = np.argsort(e2_64, kind="stable")
    emb_s = np.ascontiguousarray(emb[perm])
    e2_s = e2_64[perm]
    cmap = _colmap()
    _CACHE["perm"] = perm
    _CACHE["emb_s"] = emb_s
    _CACHE["e2_s"] = e2_s
    _CACHE["colmap"] = cmap
    _CACHE["e2min_g"] = e2_s[cmap].min(axis=1)             # [N/G]

    import ml_dtypes
    in_dt = ml_dtypes.float8_e4m3fn if MM_MODE == "fp8dr" else np.float16
    s = np.sqrt(2.0, dtype=np.float32)
    embT = np.ascontiguousarray((emb_s * s).astype(in_dt).T.reshape(2, 128, N))
    subT = np.ascontiguousarray((sub * s).astype(in_dt).T.reshape(2, 128, MQ))
    maps = []
    for c in range(NCORES):
        sl = slice(c * NSH, (c + 1) * NSH)
        maps.append({
            "embT": np.ascontiguousarray(embT[:, :, sl]),
            "subT": subT,
        })
    return maps


def _device_gmax(emb: np.ndarray, sub: np.ndarray):
    from concourse.bass_utils import run_bass_kernel_spmd
    if "nc" not in _CACHE:
        _CACHE["nc"] = _build_bass()
    nc = _CACHE["nc"]
    in_maps = _prep_inputs(emb, sub)
    _CACHE["in_maps"] = in_maps
    trace = bool(int(os.environ.get("KNN_TRACE", "0")))
    try:
        res = run_bass_kernel_spmd(nc, in_maps, core_ids=list(range(NCORES)),
                                   trace=trace)
    except ModuleNotFoundError:
        # axon NTFF profiling hook unavailable in this container
        res = run_bass_kernel_spmd(nc, in_maps, core_ids=list(range(NCORES)),
                                   trace=False)
    _CACHE["last_result"] = res
    # [8, MQ, NG] fp16 -> [MQ, N/G] f32 of max(2*dots) per device group
    V = np.concatenate([np.asarray(r["gmax"], dtype=np.float32)
                        for r in res.results], axis=1)
    return V


def _merge_host(sub, V, k):
    emb_s = _CACHE["emb_s"]
    e2_s = _CACHE["e2_s"]
    perm = _CACHE["perm"]
    cmap = _CACHE["colmap"]
    ub = V - _CACHE["e2min_g"][None, :].astype(np.float32)   # [MQ, N/G]
    _CACHE["last_vals"] = ub
    MQl = ub.shape[0]

    # top-CG groups per query by upper bound
    sel = np.argpartition(-ub, CG - 1, axis=1)[:, :CG]       # [MQ, CG]
    cols = cmap[sel].reshape(MQl, CG * G)                    # [MQ, CG*G]

    # exact rescore of candidate columns (v = 2*dots - e2, sorted space).
    # Two stages: a cheap fp32 rescore of all CG*G candidates narrows to the
    # top k+m, which are then rescored in fp64; the narrowing is safe because
    # the fp32 rescore error (~1e-3) is far below the (k+m)-th margin check.
    CB = 512                                                 # query chunk
    m = 16                                                   # fp32->fp64 slack
    e2_s32 = e2_s.astype(np.float32)
    topi = np.empty((MQl, k), dtype=np.int64)
    wk = np.empty(MQl, dtype=np.float64)
    narrow_bad = []
    for q0 in range(0, MQl, CB):
        q1 = min(q0 + CB, MQl)
        cc = cols[q0:q1]                                     # [B, CG*G]
        Ec = emb_s[cc]                                       # [B, CG*G, 256] f32
        v32 = (2.0 * np.einsum("bcd,bd->bc", Ec, sub[q0:q1],
                               dtype=np.float32)
               - e2_s32[cc])
        km = min(k + m, v32.shape[1] - 1)
        part = np.argpartition(-v32, km, axis=1)[:, :km + 1]
        ccs = np.take_along_axis(cc, part, axis=1)           # [B, km+1]
        Es = emb_s[ccs].astype(np.float64)
        vv = (2.0 * np.einsum("bcd,bd->bc", Es,
                              sub[q0:q1].astype(np.float64))
              - e2_s[ccs])
        partk = np.argpartition(-vv, k - 1, axis=1)[:, :k]
        tv = np.take_along_axis(vv, partk, axis=1)
        topi[q0:q1] = np.take_along_axis(ccs, partk, axis=1)
        wk[q0:q1] = tv.min(axis=1)
        # narrowing check: kth exact must beat the best fp32-excluded value
        # (rank km+1) by the fp32 rescore error margin
        if km + 1 < v32.shape[1]:
            excl_best = -np.partition(-v32, km + 1, axis=1)[:, km + 1]
            nb = np.nonzero(wk[q0:q1] < excl_best + 2e-2)[0] + q0
            narrow_bad.extend(nb.tolist())

    # certificate: every group that could hide a true top-k member
    # (ub >= wk - BETA) must have made the top-CG cut; also rescan any query
    # whose fp32->fp64 narrowing could not be certified
    n_close = (ub >= (wk[:, None] - BETA).astype(np.float32)).sum(axis=1)
    bad = np.nonzero(n_close > CG)[0]
    if narrow_bad:
        bad = np.union1d(bad, np.asarray(narrow_bad, dtype=np.int64))
    _CACHE["last_bad"] = bad
    if bad.size:
        # exact fp64 full rescan for uncertified queries (sorted space)
        dots = emb_s.astype(np.float64) @ sub[bad].astype(np.float64).T
        vfull = 2.0 * dots - e2_s[:, None]
        topi[bad] = np.argpartition(-vfull, k - 1, axis=0)[:k].T

    return perm[topi]                                        # original indices


def kernel(embeddings, subset, k):
    emb = np.ascontiguousarray(np.asarray(embeddings, dtype=np.float32))
    sub = np.ascontiguousarray(np.asarray(subset, dtype=np.float32))
    kk = int(np.asarray(k))
    if emb.shape != (N, D) or sub.shape != (MQ, D) or not (1 <= kk <= KMAX):
        # off-spec shapes: exact numpy fallback
        e2 = (emb * emb).sum(-1)
        dist = e2[:, None] - 2.0 * (emb @ sub.T)
        idx = np.argsort(dist, axis=0, kind="stable")[:kk].T
        return emb[idx]

    V = _device_gmax(emb, sub)
    topi = _merge_host(sub, V, kk)

    # Final ordering: rank the selected candidates by their fp64 (true)
    # distances; stable sort with index-ascending base order matches
    # jax.lax.top_k tie handling.
    topi = np.sort(topi, axis=1)                  # idx-ascending base order
    cand = emb[topi].astype(np.float64)                            # [MQ,k,D]
    e2d = (cand * cand).sum(-1)
    d64 = e2d - 2.0 * np.einsum("qkd,qd->qk", cand, sub.astype(np.float64))
    order = np.argsort(d64, axis=1, kind="stable")
    topi = np.take_along_axis(topi, order, axis=1)
    _CACHE["last_topi"] = topi
    return emb[topi]


# revision 8
# speedup vs baseline: 1249.8442x; 1249.8442x over previous
# Trainium2 Bass kernel for DeepFeatureKNN: exact k-NN (k<=16) of 4096 queries
# against 65536 database embeddings (D=256), sharded over 8 NeuronCores
# (database dim split 8 ways; every core sees all queries).
#
# The database is pre-sorted by ||e||^2 on the host, so within any block of
# 8 consecutive (sorted) rows the e2 spread is tiny. The device only computes
# 2*dots (GEMM) and per-group maxima of 2*dots; the host forms the provable
# upper bound ub(g) = gmax2dots(g) - min_{n in g} e2(n) >= v(x) - BETA for
# x in g (v = 2*dots - e2; descending v == ascending distance), selects the
# top-CG groups per query by ub, exactly rescores their member columns
# (fp32 narrowing + fp64), and picks the true top-k. Certificate: if every
# group with ub >= (kth exact v) - BETA made the top-CG cut, the candidate
# set provably contains the true top-k; failures fall back to an exact fp64
# full rescan of that query.
#
# Device-side structure (the performance-critical part):
#  - matmul: queries stationary (128 per tile), emb columns moving (512 per
#    PSUM bank). KNN_MM=fp16: two K=128 fp16 matmuls per bank. KNN_MM=fp8dr:
#    ONE fp8-e4m3 DoubleRow K=256 matmul per bank, with redundant LDWEIGHTS
#    deleted post-Tile so the 256-col (non-FWL) weight load happens once per
#    query tile instead of once per matmul (the naive form was measured
#    SLOWER than fp16 because of serialized LDWEIGHTS).
#  - group max: DVE tensor_reduce only has a 1x uop (1 elem/cyc @0.96GHz),
#    so reducing every PSUM value on DVE alone costs ~289us and was the
#    baseline bottleneck. The reduce is split across engines per 4-bank
#    group: 'D' banks reduce directly on DVE (tensor_reduce, 1 elem/cyc);
#    'S' banks are staged PSUM f32 -> SBUF bf16 by the Scalar/ACT engine
#    (1 elem/cyc @1.2GHz) and then max-combined on DVE with tensor_tensor
#    max (2x_1P mode on bf16 = 2 elem/cyc) in a tree.
#  - group geometry: groups are parity subsets of each 8-block so that the
#    TT tree and the direct reduce produce the SAME grouping: G=4 group j
#    (j in 0,1) of block b covers sorted cols {8b+j, 8b+j+2, 8b+j+4, 8b+j+6};
#    G=2 group j (j in 0..3) covers {8b+j, 8b+j+4}; G=8 is the whole block.

import os
import numpy as np

N, D, MQ = 65536, 256, 4096
NCORES = 8
NSH = N // NCORES       # 8192 database rows per core
QT = 128                # queries per partition tile
NT = 512                # database columns per tile (= one PSUM bank of fp32)
NTILES = NSH // NT      # 16
QTILES = MQ // QT       # 32
BG = 4                  # psum banks per bank-group (reduce granularity)
NBQ = NTILES // BG      # 4 bank-groups per query tile
KMAX = 16

# ---- configuration (env-overridable for A/B perf runs) ----
MM_MODE = os.environ.get("KNN_MM", "fp8dr")       # fp16 | fp8dr
G = int(os.environ.get("KNN_G", "4"))             # 8, 4, or 2 (cols/group)
# per-qt bank-group modes, comma-separated patterns cycled over qt:
# 'D' = DVE direct tensor_reduce, 'S' = ACT stage + DVE TT-max tree
PAT = os.environ.get("KNN_PAT", "DSSS,DDSS").split(",")
SPLIT_DMA = os.environ.get("KNN_SPLITDMA", "1") == "1"
DEDUP_LDW = os.environ.get("KNN_DEDUP", "1") == "1"

NG = NSH // G           # groups per core
PG = 8 // G             # groups per 8-block
if MM_MODE == "fp8dr":
    CGBASE = 80         # groups-of-8-equivalent rescored per query
    BETA = 9.0          # certificate band (device underestimate bound)
else:
    CGBASE = 48
    BETA = 0.35
CG = int(os.environ.get("KNN_CG", str(CGBASE * PG)))

_CACHE = {}


def _build_bass(repeat=None, mm=None, g=None, pat=None, dedup=None):
    import concourse.bacc as bacc
    import concourse.tile as tile
    import concourse.mybir as mybir

    mm = MM_MODE if mm is None else mm
    g = G if g is None else g
    pat = PAT if pat is None else pat
    dedup = DEDUP_LDW if dedup is None else dedup
    pg = 8 // g
    ng = NSH // g

    f32 = mybir.dt.float32
    f16 = mybir.dt.float16
    bf16 = mybir.dt.bfloat16
    in_dt = mybir.dt.float8e4 if mm == "fp8dr" else f16
    DR = mybir.MatmulPerfMode.DoubleRow
    X = mybir.AxisListType.X
    MAX = mybir.AluOpType.max

    if repeat is None:
        repeat = int(os.environ.get("KNN_REPEAT", "1"))

    nc = bacc.Bacc("TRN2", target_bir_lowering=False, debug=False,
                   num_devices=NCORES)

    embT_d = nc.dram_tensor("embT", [2, 128, NSH], in_dt, kind="ExternalInput")
    subT_d = nc.dram_tensor("subT", [2, 128, MQ], in_dt, kind="ExternalInput")
    gmax_d = nc.dram_tensor("gmax", [MQ, ng], f16, kind="ExternalOutput")

    GOUT_BQ = BG * (NT // 8) * pg   # group-max outputs per bank-group

    with tile.TileContext(nc) as tc:
        with (
            tc.tile_pool(name="const", bufs=1) as const,
            tc.tile_pool(name="stg", bufs=3) as stg,
            tc.tile_pool(name="gmp", bufs=3) as gmp,
            tc.tile_pool(name="ps", bufs=2, space="PSUM") as psum,
        ):
            subT = const.tile([128, 2, MQ], in_dt)
            if SPLIT_DMA:
                # Fine-grained input loads spread over the DGE queues so the
                # first matmuls start ~6us in instead of waiting ~23us for
                # one monolithic transfer.
                engs = [nc.sync, nc.scalar, nc.gpsimd]
                ei = 0
                for c in range(2):
                    engs[ei % 3].dma_start(subT[:, c, 0:512],
                                           subT_d[c, :, 0:512])
                    ei += 1
                embN = []
                for t in range(NTILES):
                    et = const.tile([128, 2, NT], in_dt, name=f"embN{t}")
                    embN.append(et)
                for t in range(NTILES):
                    for c in range(2):
                        sl = slice(t * NT, (t + 1) * NT)
                        engs[ei % 3].dma_start(embN[t][:, c, :],
                                               embT_d[c, :, sl])
                        ei += 1
                    if t == 3:
                        for c in range(2):
                            for h in range(1, 8):
                                sl = slice(h * 512, (h + 1) * 512)
                                engs[ei % 3].dma_start(subT[:, c, sl],
                                                       subT_d[c, :, sl])
                                ei += 1
            else:
                embT = const.tile([128, 2, NSH], in_dt)
                for c in range(2):
                    nc.sync.dma_start(embT[:, c, :], embT_d[c])
                    nc.sync.dma_start(subT[:, c, :], subT_d[c])
                embN = [embT[:, :, t * NT:(t + 1) * NT] for t in range(NTILES)]

            if repeat > 1:
                loop_cm = tc.For_i(0, repeat, 1)
            else:
                import contextlib
                loop_cm = contextlib.nullcontext()

            with loop_cm:
                for qt in range(QTILES):
                    qs = slice(qt * QT, (qt + 1) * QT)
                    qpat = pat[qt % len(pat)]
                    gm = gmp.tile([128, NBQ, GOUT_BQ], f16, tag="gm")
                    for bq in range(NBQ):
                        ps = psum.tile([128, BG, NT], f32, tag="ps")
                        if mm == "fp8dr":
                            # one DoubleRow K=256 matmul per bank; LDWEIGHTS
                            # dedup below makes the weight load once-per-qt
                            for j in range(BG):
                                nt = bq * BG + j
                                nc.tensor.matmul(ps[:, j], subT[:, :, qs],
                                                 embN[nt][:, :, :],
                                                 start=True, stop=True,
                                                 perf_mode=DR)
                        else:
                            # group matmuls by stationary operand (weight
                            # reuse): chunk 0 into all BG banks, then chunk 1
                            for c in range(2):
                                for j in range(BG):
                                    nt = bq * BG + j
                                    nc.tensor.matmul(ps[:, j], subT[:, c, qs],
                                                     embN[nt][:, c, :],
                                                     start=(c == 0),
                                                     stop=(c == 1))
                        mode = qpat[bq % len(qpat)]
                        gout = gm[:, bq].rearrange(
                            "p (b n j) -> p b n j", b=BG, n=NT // 8, j=pg)
                        if mode == "D":
                            # DVE direct reduce; strided views make parity
                            # groups that match the 'S' path's TT tree:
                            # group j of a block = cols {j, j+PG, j+2PG, ...}
                            src = ps[:].rearrange(
                                "p b (n k j) -> p b n j k", j=pg, k=g)
                            nc.vector.tensor_reduce(gout, src, axis=X, op=MAX)
                        else:
                            # ACT stages psum f32 -> SBUF bf16 (1 elem/cyc
                            # @1.2GHz), then DVE TT-max tree at 2 elem/cyc.
                            st = stg.tile([128, BG, NT // 8, 8], f16,
                                          tag="st")
                            nc.scalar.copy(st[:], ps[:])
                            if g == 2:
                                nc.vector.tensor_max(
                                    gout, st[:, :, :, 0:4], st[:, :, :, 4:8])
                            else:
                                t1 = stg.tile([128, BG, NT // 8, 4], f16,
                                              tag="t1")
                                nc.vector.tensor_max(
                                    t1[:], st[:, :, :, 0:4], st[:, :, :, 4:8])
                                if g == 4:
                                    nc.vector.tensor_max(
                                        gout, t1[:, :, :, 0:2],
                                        t1[:, :, :, 2:4])
                                else:  # G == 8
                                    t2 = stg.tile([128, BG, NT // 8, 2],
                                                  f16, tag="t2")
                                    nc.vector.tensor_max(
                                        t2[:], t1[:, :, :, 0:2],
                                        t1[:, :, :, 2:4])
                                    nc.vector.tensor_max(
                                        gout, t2[:, :, :, 0:1],
                                        t2[:, :, :, 1:2])
                    nc.sync.dma_start(gmax_d[qt * QT:(qt + 1) * QT, :], gm[:])
    if dedup:
        _dedup_ldweights(nc)
    nc.compile()
    return nc


def _dedup_ldweights(nc):
    """Delete InstLdweights whose weights signature matches the immediately
    preceding load (the PE array keeps the stationary operand resident).
    Only sync-free duplicates are removed; tracking resets at basic-block
    boundaries. This turns fp8 DoubleRow's per-matmul 256-col (non-FWL)
    weight reload into a once-per-query-tile load."""
    import concourse.mybir as mybir

    def wkey(i):
        ap = i.ins[0]
        return (repr(ap), getattr(i, "perf_mode", None),
                getattr(i, "tile_position", None),
                getattr(i, "tile_size", None))

    removed = 0
    for blk in nc.main_func.blocks:
        last = None
        keep = []
        for inst in blk.instructions:
            tn = type(inst).__name__
            if tn == "InstLdweights":
                si = inst.sync_info
                clean = si is None or (not si.on_wait and not si.on_update)
                k = wkey(inst)
                if clean and last is not None and k == last:
                    removed += 1
                    continue
                last = k
            keep.append(inst)
        if removed:
            del blk.instructions[:]
            for inst in keep:
                blk.instructions.append(inst)
    return removed


def _colmap():
    """[N//G, G] int64: sorted-space column indices of each device group."""
    blocks = np.arange(N // 8, dtype=np.int64)[:, None, None]    # [NB,1,1]
    j = np.arange(PG, dtype=np.int64)[None, :, None]             # [1,PG,1]
    k = np.arange(G, dtype=np.int64)[None, None, :]              # [1,1,G]
    cols = 8 * blocks + j + PG * k                               # [NB,PG,G]
    return cols.reshape(N // G, G)


def _prep_inputs(emb: np.ndarray, sub: np.ndarray, mm=None):
    mm = MM_MODE if mm is None else mm
    # sort database rows by ||e||^2 so each 8-block has ~zero e2 spread
    e2_64 = (emb.astype(np.float64) ** 2).sum(-1)          # [N]
    perm = np.argsort(e2_64, kind="stable")
    emb_s = np.ascontiguousarray(emb[perm])
    e2_s = e2_64[perm]
    cmap = _colmap()
    _CACHE["perm"] = perm
    _CACHE["emb_s"] = emb_s
    _CACHE["e2_s"] = e2_s
    _CACHE["colmap"] = cmap
    _CACHE["e2min_g"] = e2_s[cmap].min(axis=1)             # [N/G]

    import ml_dtypes
    in_dt = ml_dtypes.float8_e4m3fn if mm == "fp8dr" else np.float16
    s = np.sqrt(2.0, dtype=np.float32)
    embT = np.ascontiguousarray((emb_s * s).astype(in_dt).T.reshape(2, 128, N))
    subT = np.ascontiguousarray((sub * s).astype(in_dt).T.reshape(2, 128, MQ))
    maps = []
    for c in range(NCORES):
        sl = slice(c * NSH, (c + 1) * NSH)
        maps.append({
            "embT": np.ascontiguousarray(embT[:, :, sl]),
            "subT": subT,
        })
    return maps


def _device_gmax(emb: np.ndarray, sub: np.ndarray):
    from concourse.bass_utils import run_bass_kernel_spmd
    if "nc" not in _CACHE:
        _CACHE["nc"] = _build_bass()
    nc = _CACHE["nc"]
    in_maps = _prep_inputs(emb, sub)
    _CACHE["in_maps"] = in_maps
    trace = bool(int(os.environ.get("KNN_TRACE", "0")))
    try:
        res = run_bass_kernel_spmd(nc, in_maps, core_ids=list(range(NCORES)),
                                   trace=trace)
    except ModuleNotFoundError:
        # axon NTFF profiling hook unavailable in this container
        res = run_bass_kernel_spmd(nc, in_maps, core_ids=list(range(NCORES)),
                                   trace=False)
    _CACHE["last_result"] = res
    # [8, MQ, NG] fp16 -> [MQ, N/G] f32 of max(2*dots) per device group
    V = np.concatenate([np.asarray(r["gmax"], dtype=np.float32)
                        for r in res.results], axis=1)
    return V


def _merge_host(sub, V, k):
    emb_s = _CACHE["emb_s"]
    e2_s = _CACHE["e2_s"]
    perm = _CACHE["perm"]
    cmap = _CACHE["colmap"]
    ub = V - _CACHE["e2min_g"][None, :].astype(np.float32)   # [MQ, N/G]
    _CACHE["last_vals"] = ub
    MQl = ub.shape[0]

    # top-CG groups per query by upper bound
    sel = np.argpartition(-ub, CG - 1, axis=1)[:, :CG]       # [MQ, CG]
    cols = cmap[sel].reshape(MQl, CG * G)                    # [MQ, CG*G]

    # exact rescore of candidate columns (v = 2*dots - e2, sorted space).
    # Two stages: a cheap fp32 rescore of all CG*G candidates narrows to the
    # top k+m, which are then rescored in fp64; the narrowing is safe because
    # the fp32 rescore error (~1e-3) is far below the (k+m)-th margin check.
    CB = 512                                                 # query chunk
    m = 16                                                   # fp32->fp64 slack
    e2_s32 = e2_s.astype(np.float32)
    topi = np.empty((MQl, k), dtype=np.int64)
    wk = np.empty(MQl, dtype=np.float64)
    narrow_bad = []
    for q0 in range(0, MQl, CB):
        q1 = min(q0 + CB, MQl)
        cc = cols[q0:q1]                                     # [B, CG*G]
        Ec = emb_s[cc]                                       # [B, CG*G, 256] f32
        v32 = (2.0 * np.einsum("bcd,bd->bc", Ec, sub[q0:q1],
                               dtype=np.float32)
               - e2_s32[cc])
        km = min(k + m, v32.shape[1] - 1)
        part = np.argpartition(-v32, km, axis=1)[:, :km + 1]
        ccs = np.take_along_axis(cc, part, axis=1)           # [B, km+1]
        Es = emb_s[ccs].astype(np.float64)
        vv = (2.0 * np.einsum("bcd,bd->bc", Es,
                              sub[q0:q1].astype(np.float64))
              - e2_s[ccs])
        partk = np.argpartition(-vv, k - 1, axis=1)[:, :k]
        tv = np.take_along_axis(vv, partk, axis=1)
        topi[q0:q1] = np.take_along_axis(ccs, partk, axis=1)
        wk[q0:q1] = tv.min(axis=1)
        # narrowing check: kth exact must beat the best fp32-excluded value
        # (rank km+1) by the fp32 rescore error margin
        if km + 1 < v32.shape[1]:
            excl_best = -np.partition(-v32, km + 1, axis=1)[:, km + 1]
            nb = np.nonzero(wk[q0:q1] < excl_best + 2e-2)[0] + q0
            narrow_bad.extend(nb.tolist())

    # certificate: every group that could hide a true top-k member
    # (ub >= wk - BETA) must have made the top-CG cut; also rescan any query
    # whose fp32->fp64 narrowing could not be certified
    n_close = (ub >= (wk[:, None] - BETA).astype(np.float32)).sum(axis=1)
    bad = np.nonzero(n_close > CG)[0]
    if narrow_bad:
        bad = np.union1d(bad, np.asarray(narrow_bad, dtype=np.int64))
    _CACHE["last_bad"] = bad
    if bad.size:
        # exact fp64 full rescan for uncertified queries (sorted space)
        dots = emb_s.astype(np.float64) @ sub[bad].astype(np.float64).T
        vfull = 2.0 * dots - e2_s[:, None]
        topi[bad] = np.argpartition(-vfull, k - 1, axis=0)[:k].T

    return perm[topi]                                        # original indices


def kernel(embeddings, subset, k):
    emb = np.ascontiguousarray(np.asarray(embeddings, dtype=np.float32))
    sub = np.ascontiguousarray(np.asarray(subset, dtype=np.float32))
    kk = int(np.asarray(k))
    if emb.shape != (N, D) or sub.shape != (MQ, D) or not (1 <= kk <= KMAX):
        # off-spec shapes: exact numpy fallback
        e2 = (emb * emb).sum(-1)
        dist = e2[:, None] - 2.0 * (emb @ sub.T)
        idx = np.argsort(dist, axis=0, kind="stable")[:kk].T
        return emb[idx]

    V = _device_gmax(emb, sub)
    topi = _merge_host(sub, V, kk)

    # Final ordering: rank the selected candidates by their fp64 (true)
    # distances; stable sort with index-ascending base order matches
    # jax.lax.top_k tie handling.
    topi = np.sort(topi, axis=1)                  # idx-ascending base order
    cand = emb[topi].astype(np.float64)                            # [MQ,k,D]
    e2d = (cand * cand).sum(-1)
    d64 = e2d - 2.0 * np.einsum("qkd,qd->qk", cand, sub.astype(np.float64))
    order = np.argsort(d64, axis=1, kind="stable")
    topi = np.take_along_axis(topi, order, axis=1)
    _CACHE["last_topi"] = topi
    return emb[topi]
